# revision 26
# baseline (speedup 1.0000x reference)
"""Grouped MLP (8-expert SwiGLU) Trainium2 Bass kernel, v2 (bf16 PE path).

Sharding: expert-parallel, one group per NeuronCore (8 cores).
Token t belongs to group t % 8, so core n gets x[n::8] (4096 tokens),
its expert's gate/up/down weights, and produces out[n::8].

v2 changes over the fp32r baseline (726.3us):
- All matmuls in bf16: the PE streams bf16 moving data at 1.0 cycle/row
  (512-row MM spacing measured 215.8ns) while fp32r pays a 17/16 row tax
  (226.7ns).  NOTE: float16 is NOT usable here - kernels with a large
  fp16 matmul count get statically downclocked to 2.0GHz chip-wide
  (259ns/MM); bf16 keeps the full 2.4GHz.  Accumulation stays fp32 in
  PSUM; end-to-end error vs the fp32 reference is 4.1e-3 (gate 2e-2).
- Host pre-packs every tensor into the exact SBUF tile layout, so each
  DMA is one fully contiguous read (2KB+ per partition row, line rate).
- Dual HWDGE rings: gate weights + output writes issue on the sync
  queue, up weights + activations (+half of wd) on the scalar queue.
  The two rings drain independently, halving the startup serial chain.
- PE warm-up: a 14-MM dummy chain on a zeroed scratch tile issues as
  soon as the engine barrier clears (~7.2us), so the HAM clock gate
  (cold 1.2GHz -> warm 2.4GHz after ~3.4us of activity) is already warm
  when the first real chain's data lands; block 0's first chunk then
  ramps in 256-wide t-tiles so the PE starts on partial xt.
- Cross-block prefetch: xt and the first two weight chunk-pairs of
  block tb+1 issue before block tb's down-projection loop, so the PE
  never waits at a block boundary.
"""

import sys

if "/opt/trn_rl_repo" not in sys.path:
    sys.path.insert(0, "/opt/trn_rl_repo")

import ml_dtypes
import numpy as np

import concourse.bass as bass  # noqa: F401  (registers bass machinery)
import concourse.tile as tile
from concourse import bacc, mybir
from concourse.bass_utils import run_bass_kernel_spmd

P = 128
T = 4096   # tokens per core (per group)
K = 1024   # d_in
H = 2048   # d_hid
O = 1024   # d_out
N_CORES = 8

F16 = mybir.dt.bfloat16
F32 = mybir.dt.float32

# Tiling knobs
TB = 1024           # token block
WCH = 128           # gate/up weight chunk width along hidden dim
MMF = 512           # matmul moving free dim (one fp32 PSUM bank)

KO = K // P         # 8  k-subtiles
HO = H // P         # 16 h-subtiles
NTB = T // TB       # token blocks
NT = TB // MMF      # 512-wide t-tiles per token block
NWC = H // WCH      # weight chunks per block
NO = O // MMF       # 512-wide o-tiles

_CACHED_NC = None


def _build_nc():
    from contextlib import ExitStack

    nc = bacc.Bacc(None, target_bir_lowering=False)
    # Host-packed layouts (all contiguous in the order the DMAs read them):
    #   xt:  [NTB][P][KO][TB]    x transposed + k-tiled, per token block
    #   wg/wu: [NWC][P][KO][WCH] weight chunks, k-tiled
    #   wd:  [HO][P][O]          down weights, h-tiled
    xt = nc.dram_tensor("xt", [NTB, P, KO, TB], F16, kind="ExternalInput")
    wg = nc.dram_tensor("wg", [NWC, P, KO, WCH], F16, kind="ExternalInput")
    wu = nc.dram_tensor("wu", [NWC, P, KO, WCH], F16, kind="ExternalInput")
    wd = nc.dram_tensor("wd", [HO, P, O], F16, kind="ExternalInput")
    out = nc.dram_tensor("out", [T, O], F32, kind="ExternalOutput")

    silu_fn = mybir.ActivationFunctionType.Silu

    with tile.TileContext(nc) as tc, ExitStack() as ctx:
        const = ctx.enter_context(tc.tile_pool(name="const", bufs=1))
        xpool = ctx.enter_context(tc.tile_pool(name="xpool", bufs=2))
        wpool = ctx.enter_context(tc.tile_pool(name="wpool", bufs=4))
        hpool = ctx.enter_context(tc.tile_pool(name="hpool", bufs=1))
        spool = ctx.enter_context(tc.tile_pool(name="spool", bufs=2))
        opool = ctx.enter_context(tc.tile_pool(name="opool", bufs=2))
        ps12 = ctx.enter_context(tc.tile_pool(name="ps12", bufs=2, space="PSUM"))
        ps3 = ctx.enter_context(tc.tile_pool(name="ps3", bufs=3, space="PSUM"))
        psw = ctx.enter_context(tc.tile_pool(name="psw", bufs=1, space="PSUM"))

        # --- PE warm-up: one 8-MM dummy chain on zeroed scratch. Issues
        # right after the engine barrier; data dep only on the memset.
        scratch = const.tile([P, MMF], F16)
        nc.vector.memset(scratch[:], 0)
        warm_ps = psw.tile([P, MMF], F32, tag="warm")
        for i in range(14):
            nc.tensor.matmul(
                warm_ps[:],
                scratch[:, 0:P],
                scratch[:],
                start=(i == 0),
                stop=(i == 13),
            )

        # Down-projection weights resident for the whole kernel.
        wd_sb = const.tile([P, HO, O], F16)

        # Persistent tile handles across the tb loop (allocated per tb).
        xt_tiles = {}

        def issue_xt(tb):
            xt_sb = xpool.tile([P, KO, TB], F16, tag="xt", name=f"xt{tb}")
            nc.scalar.dma_start(xt_sb[:], xt[tb])
            xt_tiles[tb] = xt_sb

        wg_tiles = {}
        wu_tiles = {}

        def issue_wg(tb, wc):
            wg_sb = wpool.tile([P, KO, WCH], F16, tag="wg", name=f"wg{tb}_{wc}")
            nc.sync.dma_start(wg_sb[:], wg[wc])
            wg_tiles[(tb, wc)] = wg_sb

        def issue_wu(tb, wc):
            wu_sb = wpool.tile([P, KO, WCH], F16, tag="wu", name=f"wu{tb}_{wc}")
            nc.scalar.dma_start(wu_sb[:], wu[wc])
            wu_tiles[(tb, wc)] = wu_sb

        # tb0 startup: first chunk-pair + xt block 0 issue first, with xt0
        # split across both rings so the serial chain is as short as
        # possible (each ring carries ~1.25MB before the first chain's
        # inputs are complete).
        # xt block 0 in quadrants across both rings; the first 256-wide
        # chains need only quadrant 0.  At kernel start all 8 cores
        # burst-saturate HBM and each ring only sustains ~180GB/s —
        # minimizing critical bytes per ring is what matters.
        issue_wg(0, 0)      # sync ring:   wg0 (first gate chain's weights)
        issue_wu(0, 0)      # scalar ring: wu0 (first up chain's weights)
        xt0_sb = xpool.tile([P, KO, TB], F16, tag="xt", name="xt0")
        for q in range(0, TB, 256):
            nc.sync.dma_start(xt0_sb[:, 0:4, q : q + 256], xt[0, :, 0:4, q : q + 256])
            nc.scalar.dma_start(xt0_sb[:, 4:8, q : q + 256], xt[0, :, 4:8, q : q + 256])
        xt_tiles[0] = xt0_sb
        issue_wg(0, 1)      # keep chunk 1 weights ahead of the wd weave
        issue_wu(0, 1)

        def gateup(wg_sb, wu_sb, h, tsl, xt_sb, hid_sb):
            mmf = tsl.stop - tsl.start
            gate_ps = ps12.tile([P, mmf], F32, tag="gate")
            for ko in range(KO):
                nc.tensor.matmul(
                    gate_ps[:],
                    wg_sb[:, ko, :],
                    xt_sb[:, ko, tsl],
                    start=(ko == 0),
                    stop=(ko == KO - 1),
                )
            up_ps = ps12.tile([P, mmf], F32, tag="up")
            for ko in range(KO):
                nc.tensor.matmul(
                    up_ps[:],
                    wu_sb[:, ko, :],
                    xt_sb[:, ko, tsl],
                    start=(ko == 0),
                    stop=(ko == KO - 1),
                )
            silu_sb = spool.tile([P, mmf], F32, tag="silu")
            nc.scalar.activation(silu_sb[:], gate_ps[:], silu_fn)
            nc.vector.tensor_mul(hid_sb[:, h, tsl], silu_sb[:], up_ps[:])

        for tb in range(NTB):
            xt_sb = xt_tiles.pop(tb)
            hid_sb = hpool.tile([P, HO, TB], F16, tag="hid")

            if tb == 0:
                # Startup ramp: chunk 0 in 256-wide tiles interleaved with
                # chunk 1's 512-wide chains, ordered so every chain's xt
                # quadrant has landed (or is about to) when the PE reaches
                # it — the PE never sits idle while xt streams in.
                wg0_sb = wg_tiles.pop((0, 0))
                wu0_sb = wu_tiles.pop((0, 0))
                wg1_sb = wg_tiles.pop((0, 1))
                wu1_sb = wu_tiles.pop((0, 1))
                def dummy_fill(n, width):
                    # Keep the HAM activity monitor fed during the ramp's
                    # HBM-bound data waits: a >~2us idle stretch trips the
                    # MID window and re-throttles the PE to 1.2GHz for
                    # 3.4us.  These dummies run only while the PE would be
                    # idle anyway (worst case they delay real work by one
                    # short chain).
                    for i in range(n):
                        nc.tensor.matmul(
                            warm_ps[:, 0:width],
                            scratch[:, 0:P],
                            scratch[:, 0:width],
                            start=(i == 0),
                            stop=(i == n - 1),
                        )

                for wpair, h, lo, hi in (
                    ((wg0_sb, wu0_sb), 0, 0, 256),
                    ((wg0_sb, wu0_sb), 0, 256, 512),
                    ((wg0_sb, wu0_sb), 0, 512, 768),
                    ((wg0_sb, wu0_sb), 0, 768, 1024),
                    ((wg1_sb, wu1_sb), 1, 0, 512),
                    ((wg1_sb, wu1_sb), 1, 512, 1024),
                ):
                    if h == 0 and lo == 512:
                        dummy_fill(10, 256)
                    gateup(wpair[0], wpair[1], h, slice(lo, hi), xt_sb, hid_sb)

            for wc in range(2 if tb == 0 else 0, NWC):
                if (tb, wc) not in wg_tiles:
                    issue_wg(tb, wc)
                if (tb, wc) not in wu_tiles:
                    issue_wu(tb, wc)
                wg_sb = wg_tiles.pop((tb, wc))
                wu_sb = wu_tiles.pop((tb, wc))
                if tb == 0 and 8 <= wc < 16:
                    # Weave the resident down-projection weights between
                    # block-0 chunks (two per chunk, one per ring), in the
                    # back half so the early weight stream is never delayed;
                    # still complete long before the down phase reads them.
                    ho2 = (wc - 8) * 2
                    nc.sync.dma_start(wd_sb[:, ho2, :], wd[ho2])
                    nc.scalar.dma_start(wd_sb[:, ho2 + 1, :], wd[ho2 + 1])

                for th in range(NT):
                    gateup(wg_sb, wu_sb, wc, slice(th * MMF, (th + 1) * MMF),
                           xt_sb, hid_sb)

            # Prefetch next block's activations + first chunk-pairs before
            # the down loop, so the PE never waits at the block boundary.
            if tb + 1 < NTB:
                issue_xt(tb + 1)
                issue_wg(tb + 1, 0)
                issue_wg(tb + 1, 1)
                issue_wu(tb + 1, 0)
                issue_wu(tb + 1, 1)

            # Down projection for this token block.  The very last output
            # tile of the kernel is computed as two 256-wide chains so the
            # tail (final copy + DMA after the last matmul) is halved.
            for ti in range(TB // P):
                for oi in range(NO):
                    last = tb == NTB - 1 and ti == TB // P - 1 and oi == NO - 1
                    for osl in (
                        [slice(oi * MMF, oi * MMF + 256),
                         slice(oi * MMF + 256, (oi + 1) * MMF)]
                        if last
                        else [slice(oi * MMF, (oi + 1) * MMF)]
                    ):
                        width = osl.stop - osl.start
                        out_ps = ps3.tile([P, width], F32, tag="outp")
                        for ho in range(HO):
                            nc.tensor.matmul(
                                out_ps[:],
                                hid_sb[:, ho, ti * P : (ti + 1) * P],
                                wd_sb[:, ho, osl],
                                start=(ho == 0),
                                stop=(ho == HO - 1),
                            )
                        ob = opool.tile([P, width], F32, tag="ob")
                        nc.vector.tensor_copy(ob[:], out_ps[:])
                        nc.sync.dma_start(
                            out[tb * TB + ti * P : tb * TB + (ti + 1) * P, osl],
                            ob[:],
                        )

    nc.compile()
    return nc


def _get_nc():
    global _CACHED_NC
    if _CACHED_NC is None:
        _CACHED_NC = _build_nc()
    return _CACHED_NC


def _pack_weights(w):
    """[K, H] -> [NWC, P, KO, WCH] fp16, contiguous."""
    # w[ko*P + p, wc*WCH + h] -> wp[wc, p, ko, h]
    w4 = w.reshape(KO, P, NWC, WCH).transpose(2, 1, 0, 3)
    return np.ascontiguousarray(w4).astype(ml_dtypes.bfloat16)


def _pack_xt(xg):
    """[T, K] tokens-of-group -> [NTB, P, KO, TB] fp16, contiguous."""
    # xg[tb*TB + t, ko*P + p] -> xp[tb, p, ko, t]
    x4 = xg.reshape(NTB, TB, KO, P).transpose(0, 3, 2, 1)
    return np.ascontiguousarray(x4).astype(ml_dtypes.bfloat16)


def _pack_wd(w):
    """[H, O] -> [HO, P, O] fp16, contiguous."""
    w3 = w.reshape(HO, P, O)
    return np.ascontiguousarray(w3).astype(ml_dtypes.bfloat16)


def _make_in_maps(x, gate_weight, up_weight, down_weight, n):
    in_maps = []
    for g in range(n):
        in_maps.append(
            {
                "xt": _pack_xt(x[g::n]),
                "wg": _pack_weights(gate_weight[g]),
                "wu": _pack_weights(up_weight[g]),
                "wd": _pack_wd(down_weight[g]),
            }
        )
    return in_maps


def _run_spmd(in_maps, **kwargs):
    nc = _get_nc()
    return run_bass_kernel_spmd(nc, in_maps, core_ids=list(range(N_CORES)), **kwargs)


def kernel(x, gate_weight, up_weight, down_weight, num_groups=8):
    n = int(num_groups)
    x = np.asarray(x, dtype=np.float32)
    gate_weight = np.asarray(gate_weight, dtype=np.float32)
    up_weight = np.asarray(up_weight, dtype=np.float32)
    down_weight = np.asarray(down_weight, dtype=np.float32)

    assert n == N_CORES, f"expected {N_CORES} groups, got {n}"
    assert x.shape == (T * N_CORES, K), x.shape
    assert gate_weight.shape == (n, K, H), gate_weight.shape
    assert up_weight.shape == (n, K, H), up_weight.shape
    assert down_weight.shape == (n, H, O), down_weight.shape

    in_maps = _make_in_maps(x, gate_weight, up_weight, down_weight, n)
    res = _run_spmd(in_maps)

    out = np.empty((x.shape[0], O), dtype=np.float32)
    for g in range(n):
        out[g::n] = res.results[g]["out"]
    return out


# revision 27
# speedup vs baseline: 1.0101x; 1.0101x over previous
"""Grouped MLP (8-expert SwiGLU) Trainium2 Bass kernel, v2 (bf16 PE path).

Sharding: expert-parallel, one group per NeuronCore (8 cores).
Token t belongs to group t % 8, so core n gets x[n::8] (4096 tokens),
its expert's gate/up/down weights, and produces out[n::8].

v2 changes over the fp32r baseline (726.3us):
- All matmuls in bf16: the PE streams bf16 moving data at 1.0 cycle/row
  (512-row MM spacing measured 215.8ns) while fp32r pays a 17/16 row tax
  (226.7ns).  NOTE: float16 is NOT usable here - kernels with a large
  fp16 matmul count get statically downclocked to 2.0GHz chip-wide
  (259ns/MM); bf16 keeps the full 2.4GHz.  Accumulation stays fp32 in
  PSUM; end-to-end error vs the fp32 reference is 4.1e-3 (gate 2e-2).
- Host pre-packs every tensor into the exact SBUF tile layout, so each
  DMA is one fully contiguous read (2KB+ per partition row, line rate).
- Dual HWDGE rings: gate weights + output writes issue on the sync
  queue, up weights + activations (+half of wd) on the scalar queue.
  The two rings drain independently, halving the startup serial chain.
- PE warm-up: a 14-MM dummy chain on a zeroed scratch tile issues as
  soon as the engine barrier clears (~7.2us), so the HAM clock gate
  (cold 1.2GHz -> warm 2.4GHz after ~3.4us of activity) is already warm
  when the first real chain's data lands; block 0's first chunk then
  ramps in 256-wide t-tiles so the PE starts on partial xt.
- Cross-block prefetch: xt and the first two weight chunk-pairs of
  block tb+1 issue before block tb's down-projection loop, so the PE
  never waits at a block boundary.
"""

import sys

if "/opt/trn_rl_repo" not in sys.path:
    sys.path.insert(0, "/opt/trn_rl_repo")

import ml_dtypes
import numpy as np

import concourse.bass as bass  # noqa: F401  (registers bass machinery)
import concourse.tile as tile
from concourse import bacc, mybir
from concourse.bass_utils import run_bass_kernel_spmd

P = 128
T = 4096   # tokens per core (per group)
K = 1024   # d_in
H = 2048   # d_hid
O = 1024   # d_out
N_CORES = 8

F16 = mybir.dt.bfloat16
F32 = mybir.dt.float32

# Tiling knobs
TB = 1024           # token block
WCH = 128           # gate/up weight chunk width along hidden dim
MMF = 512           # matmul moving free dim (one fp32 PSUM bank)

KO = K // P         # 8  k-subtiles
HO = H // P         # 16 h-subtiles
NTB = T // TB       # token blocks
NT = TB // MMF      # 512-wide t-tiles per token block
NWC = H // WCH      # weight chunks per block
NO = O // MMF       # 512-wide o-tiles

_CACHED_NC = None


def _build_nc():
    from contextlib import ExitStack

    nc = bacc.Bacc(None, target_bir_lowering=False)
    # Host-packed layouts (all contiguous in the order the DMAs read them):
    #   xt:  [NTB][P][KO][TB]    x transposed + k-tiled, per token block
    #   wg/wu: [NWC][P][KO][WCH] weight chunks, k-tiled
    #   wd:  [HO][P][O]          down weights, h-tiled
    xt = nc.dram_tensor("xt", [NTB, P, KO, TB], F16, kind="ExternalInput")
    wg = nc.dram_tensor("wg", [NWC, P, KO, WCH], F16, kind="ExternalInput")
    wu = nc.dram_tensor("wu", [NWC, P, KO, WCH], F16, kind="ExternalInput")
    wd = nc.dram_tensor("wd", [HO, P, O], F16, kind="ExternalInput")
    out = nc.dram_tensor("out", [T, O], F32, kind="ExternalOutput")

    silu_fn = mybir.ActivationFunctionType.Silu

    with tile.TileContext(nc) as tc, ExitStack() as ctx:
        const = ctx.enter_context(tc.tile_pool(name="const", bufs=1))
        xpool = ctx.enter_context(tc.tile_pool(name="xpool", bufs=2))
        wpool = ctx.enter_context(tc.tile_pool(name="wpool", bufs=4))
        hpool = ctx.enter_context(tc.tile_pool(name="hpool", bufs=1))
        spool = ctx.enter_context(tc.tile_pool(name="spool", bufs=2))
        opool = ctx.enter_context(tc.tile_pool(name="opool", bufs=2))
        ps12 = ctx.enter_context(tc.tile_pool(name="ps12", bufs=2, space="PSUM"))
        ps3 = ctx.enter_context(tc.tile_pool(name="ps3", bufs=3, space="PSUM"))
        psw = ctx.enter_context(tc.tile_pool(name="psw", bufs=1, space="PSUM"))

        # --- PE warm-up: one 8-MM dummy chain on zeroed scratch. Issues
        # right after the engine barrier; data dep only on the memset.
        scratch = const.tile([P, MMF], F16)
        nc.vector.memset(scratch[:], 0)
        warm_ps = psw.tile([P, MMF], F32, tag="warm")
        for i in range(14):
            nc.tensor.matmul(
                warm_ps[:],
                scratch[:, 0:P],
                scratch[:],
                start=(i == 0),
                stop=(i == 13),
            )

        # Down-projection weights resident for the whole kernel.
        wd_sb = const.tile([P, HO, O], F16)

        # Persistent tile handles across the tb loop (allocated per tb).
        xt_tiles = {}

        def issue_xt(tb):
            xt_sb = xpool.tile([P, KO, TB], F16, tag="xt", name=f"xt{tb}")
            nc.scalar.dma_start(xt_sb[:], xt[tb])
            xt_tiles[tb] = xt_sb

        wg_tiles = {}
        wu_tiles = {}

        def issue_wg(tb, wc):
            wg_sb = wpool.tile([P, KO, WCH], F16, tag="wg", name=f"wg{tb}_{wc}")
            nc.sync.dma_start(wg_sb[:], wg[wc])
            wg_tiles[(tb, wc)] = wg_sb

        def issue_wu(tb, wc):
            wu_sb = wpool.tile([P, KO, WCH], F16, tag="wu", name=f"wu{tb}_{wc}")
            nc.scalar.dma_start(wu_sb[:], wu[wc])
            wu_tiles[(tb, wc)] = wu_sb

        # tb0 startup: first chunk-pair + xt block 0 issue first, with xt0
        # split across both rings so the serial chain is as short as
        # possible (each ring carries ~1.25MB before the first chain's
        # inputs are complete).
        # xt block 0 in quadrants across both rings; the first 256-wide
        # chains need only quadrant 0.  At kernel start all 8 cores
        # burst-saturate HBM and each ring only sustains ~180GB/s —
        # minimizing critical bytes per ring is what matters.
        issue_wg(0, 0)      # sync ring:   wg0 (first gate chain's weights)
        issue_wu(0, 0)      # scalar ring: wu0 (first up chain's weights)
        xt0_sb = xpool.tile([P, KO, TB], F16, tag="xt", name="xt0")
        for q in range(0, TB, 256):
            nc.sync.dma_start(xt0_sb[:, 0:4, q : q + 256], xt[0, :, 0:4, q : q + 256])
            nc.scalar.dma_start(xt0_sb[:, 4:8, q : q + 256], xt[0, :, 4:8, q : q + 256])
        xt_tiles[0] = xt0_sb
        issue_wg(0, 1)      # keep chunk 1 weights ahead of the wd weave
        issue_wu(0, 1)

        def gateup(wg_sb, wu_sb, h, tsl, xt_sb, hid_sb):
            mmf = tsl.stop - tsl.start
            gate_ps = ps12.tile([P, mmf], F32, tag="gate")
            for ko in range(KO):
                nc.tensor.matmul(
                    gate_ps[:],
                    wg_sb[:, ko, :],
                    xt_sb[:, ko, tsl],
                    start=(ko == 0),
                    stop=(ko == KO - 1),
                )
            up_ps = ps12.tile([P, mmf], F32, tag="up")
            for ko in range(KO):
                nc.tensor.matmul(
                    up_ps[:],
                    wu_sb[:, ko, :],
                    xt_sb[:, ko, tsl],
                    start=(ko == 0),
                    stop=(ko == KO - 1),
                )
            silu_sb = spool.tile([P, mmf], F32, tag="silu")
            nc.scalar.activation(silu_sb[:], gate_ps[:], silu_fn)
            nc.vector.tensor_mul(hid_sb[:, h, tsl], silu_sb[:], up_ps[:])

        for tb in range(NTB):
            xt_sb = xt_tiles.pop(tb)
            hid_sb = hpool.tile([P, HO, TB], F16, tag="hid")

            if tb == 0:
                # Startup ramp: chunk 0 in 256-wide tiles interleaved with
                # chunk 1's 512-wide chains, ordered so every chain's xt
                # quadrant has landed (or is about to) when the PE reaches
                # it — the PE never sits idle while xt streams in.
                wg0_sb = wg_tiles.pop((0, 0))
                wu0_sb = wu_tiles.pop((0, 0))
                wg1_sb = wg_tiles.pop((0, 1))
                wu1_sb = wu_tiles.pop((0, 1))
                for wpair, h, lo, hi in (
                    ((wg0_sb, wu0_sb), 0, 0, 256),
                    ((wg0_sb, wu0_sb), 0, 256, 512),
                    ((wg0_sb, wu0_sb), 0, 512, 768),
                    ((wg0_sb, wu0_sb), 0, 768, 1024),
                    ((wg1_sb, wu1_sb), 1, 0, 512),
                    ((wg1_sb, wu1_sb), 1, 512, 1024),
                ):
                    gateup(wpair[0], wpair[1], h, slice(lo, hi), xt_sb, hid_sb)

            for wc in range(2 if tb == 0 else 0, NWC):
                if (tb, wc) not in wg_tiles:
                    issue_wg(tb, wc)
                if (tb, wc) not in wu_tiles:
                    issue_wu(tb, wc)
                wg_sb = wg_tiles.pop((tb, wc))
                wu_sb = wu_tiles.pop((tb, wc))
                if tb == 0 and 8 <= wc < 16:
                    # Weave the resident down-projection weights between
                    # block-0 chunks (two per chunk, one per ring), in the
                    # back half so the early weight stream is never delayed;
                    # still complete long before the down phase reads them.
                    ho2 = (wc - 8) * 2
                    nc.sync.dma_start(wd_sb[:, ho2, :], wd[ho2])
                    nc.scalar.dma_start(wd_sb[:, ho2 + 1, :], wd[ho2 + 1])

                for th in range(NT):
                    gateup(wg_sb, wu_sb, wc, slice(th * MMF, (th + 1) * MMF),
                           xt_sb, hid_sb)

            # Prefetch next block's activations + first chunk-pairs before
            # the down loop, so the PE never waits at the block boundary.
            if tb + 1 < NTB:
                issue_xt(tb + 1)
                issue_wg(tb + 1, 0)
                issue_wg(tb + 1, 1)
                issue_wu(tb + 1, 0)
                issue_wu(tb + 1, 1)

            # Down projection for this token block.  The very last output
            # tile of the kernel is computed as two 256-wide chains so the
            # tail (final copy + DMA after the last matmul) is halved.
            for ti in range(TB // P):
                for oi in range(NO):
                    last = tb == NTB - 1 and ti == TB // P - 1 and oi == NO - 1
                    for osl in (
                        [slice(oi * MMF, oi * MMF + 256),
                         slice(oi * MMF + 256, (oi + 1) * MMF)]
                        if last
                        else [slice(oi * MMF, (oi + 1) * MMF)]
                    ):
                        width = osl.stop - osl.start
                        out_ps = ps3.tile([P, width], F32, tag="outp")
                        for ho in range(HO):
                            nc.tensor.matmul(
                                out_ps[:],
                                hid_sb[:, ho, ti * P : (ti + 1) * P],
                                wd_sb[:, ho, osl],
                                start=(ho == 0),
                                stop=(ho == HO - 1),
                            )
                        ob = opool.tile([P, width], F32, tag="ob")
                        nc.vector.tensor_copy(ob[:], out_ps[:])
                        nc.sync.dma_start(
                            out[tb * TB + ti * P : tb * TB + (ti + 1) * P, osl],
                            ob[:],
                        )

    nc.compile()
    return nc


def _get_nc():
    global _CACHED_NC
    if _CACHED_NC is None:
        _CACHED_NC = _build_nc()
    return _CACHED_NC


def _pack_weights(w):
    """[K, H] -> [NWC, P, KO, WCH] fp16, contiguous."""
    # w[ko*P + p, wc*WCH + h] -> wp[wc, p, ko, h]
    w4 = w.reshape(KO, P, NWC, WCH).transpose(2, 1, 0, 3)
    return np.ascontiguousarray(w4).astype(ml_dtypes.bfloat16)


def _pack_xt(xg):
    """[T, K] tokens-of-group -> [NTB, P, KO, TB] fp16, contiguous."""
    # xg[tb*TB + t, ko*P + p] -> xp[tb, p, ko, t]
    x4 = xg.reshape(NTB, TB, KO, P).transpose(0, 3, 2, 1)
    return np.ascontiguousarray(x4).astype(ml_dtypes.bfloat16)


def _pack_wd(w):
    """[H, O] -> [HO, P, O] fp16, contiguous."""
    w3 = w.reshape(HO, P, O)
    return np.ascontiguousarray(w3).astype(ml_dtypes.bfloat16)


def _make_in_maps(x, gate_weight, up_weight, down_weight, n):
    in_maps = []
    for g in range(n):
        in_maps.append(
            {
                "xt": _pack_xt(x[g::n]),
                "wg": _pack_weights(gate_weight[g]),
                "wu": _pack_weights(up_weight[g]),
                "wd": _pack_wd(down_weight[g]),
            }
        )
    return in_maps


def _run_spmd(in_maps, **kwargs):
    nc = _get_nc()
    return run_bass_kernel_spmd(nc, in_maps, core_ids=list(range(N_CORES)), **kwargs)


def kernel(x, gate_weight, up_weight, down_weight, num_groups=8):
    n = int(num_groups)
    x = np.asarray(x, dtype=np.float32)
    gate_weight = np.asarray(gate_weight, dtype=np.float32)
    up_weight = np.asarray(up_weight, dtype=np.float32)
    down_weight = np.asarray(down_weight, dtype=np.float32)

    assert n == N_CORES, f"expected {N_CORES} groups, got {n}"
    assert x.shape == (T * N_CORES, K), x.shape
    assert gate_weight.shape == (n, K, H), gate_weight.shape
    assert up_weight.shape == (n, K, H), up_weight.shape
    assert down_weight.shape == (n, H, O), down_weight.shape

    in_maps = _make_in_maps(x, gate_weight, up_weight, down_weight, n)
    res = _run_spmd(in_maps)

    out = np.empty((x.shape[0], O), dtype=np.float32)
    for g in range(n):
        out[g::n] = res.results[g]["out"]
    return out
